# revision 4
# baseline (speedup 1.0000x reference)
"""Trainium2 Bass kernel for ExpertGatedAggregator (moe_routing).

Data-parallel over batch across 8 NeuronCores. Per core:
  - cast expert_reprs fp32 -> bf16 via SWDGE compute-DMA (DRAM->DRAM)
  - xbar transpose-DMA bf16 tiles into feature-major SBUF layout
  - 3-layer gate MLP on PE with weights stationary (bf16 in, fp32 accum)
  - masked-softmax renorm epilogue on ACT/DVE
"""
import numpy as np

import concourse.bacc as bacc
import concourse.mybir as mybir
from concourse import tile
from concourse.bass_utils import run_bass_kernel_spmd

E, B, D, H = 8, 32768, 256, 1024
NCORES = 8
BC = B // NCORES      # 4096 batch rows per core
NB = 512              # batch tile (matmul free dim, one PSUM bank)
NT = BC // NB         # 8 batch tiles per core
NS = NB // 128        # 4 sub-tiles of 128 rows per batch tile
KC = (E * D) // 128   # 16 feature chunks of 128
HT = H // 128         # 8 H-tiles (layer-1 output)
H2 = H // 2           # 512
H2T = H2 // 128       # 4 tiles (layer-2 output)

f32 = mybir.dt.float32
bf16 = mybir.dt.bfloat16

_CACHE = {}


def _build_program(nt=NT, skip=()):
    nc = bacc.Bacc("TRN2", target_bir_lowering=False, debug=False,
                   num_devices=NCORES)

    xr = nc.declare_dram_parameter("xr", [E, BC, D], f32, isOutput=False)
    xp = nc.declare_dram_parameter("xp", [E, BC], f32, isOutput=False)
    am = nc.declare_dram_parameter("am", [BC, E], f32, isOutput=False)
    w1 = nc.declare_dram_parameter("w1", [E * D + E, H], f32, isOutput=False)
    b1 = nc.declare_dram_parameter("b1", [H], f32, isOutput=False)
    w2 = nc.declare_dram_parameter("w2", [H, H2], f32, isOutput=False)
    b2 = nc.declare_dram_parameter("b2", [H2], f32, isOutput=False)
    w3 = nc.declare_dram_parameter("w3", [H2, E], f32, isOutput=False)
    b3 = nc.declare_dram_parameter("b3", [E], f32, isOutput=False)
    ident = nc.declare_dram_parameter("ident", [E, E], f32, isOutput=False)
    wp_o = nc.declare_dram_parameter("wp", [BC, 1], f32, isOutput=True)
    gw_o = nc.declare_dram_parameter("gw", [BC, E], f32, isOutput=True)

    xbf = nc.dram_tensor("xbf", [E, BC, D], bf16)  # bf16 bounce buffer

    with tile.TileContext(nc) as tc:
        with (
            tc.tile_pool(name="wpool", bufs=1) as wpool,
            tc.tile_pool(name="xtp", bufs=2) as xtp,
            tc.tile_pool(name="h1p", bufs=2) as h1p,
            tc.tile_pool(name="h2p", bufs=2) as h2p,
            tc.tile_pool(name="zp", bufs=2) as zp,
            tc.tile_pool(name="ep", bufs=4) as ep,
            tc.tile_pool(name="op", bufs=2) as op,
            tc.tile_pool(name="ps1", bufs=3, space="PSUM") as ps1p,
            tc.tile_pool(name="ps2", bufs=2, space="PSUM") as ps2p,
            tc.tile_pool(name="ps3", bufs=1, space="PSUM") as ps3p,
            tc.tile_pool(name="psz", bufs=1, space="PSUM") as pszp,
            tc.tile_pool(name="psp", bufs=1, space="PSUM") as pspp,
        ):
            # ---- weights / constants (resident) ----
            w1sb = wpool.tile([128, KC, H], bf16, tag="w1sb")
            nc.gpsimd.dma_start(
                w1sb[:], w1.ap()[: E * D].rearrange("(k p) h -> p k h", p=128))
            w1pr = wpool.tile([E, H], bf16, tag="w1pr")
            nc.gpsimd.dma_start(w1pr[:], w1.ap()[E * D:])
            w2sb = wpool.tile([128, HT, H2], bf16, tag="w2sb")
            nc.gpsimd.dma_start(
                w2sb[:], w2.ap().rearrange("(k p) h -> p k h", p=128))
            w3sb = wpool.tile([128, H2T, E], bf16, tag="w3sb")
            nc.gpsimd.dma_start(
                w3sb[:], w3.ap().rearrange("(k p) h -> p k h", p=128))

            b1sb = wpool.tile([128, HT], f32, tag="b1sb")
            for j in range(HT):
                nc.sync.dma_start(b1sb[:, j : j + 1], b1.ap()[j * 128:(j + 1) * 128])
            b2sb = wpool.tile([128, H2T], f32, tag="b2sb")
            for m in range(H2T):
                nc.sync.dma_start(b2sb[:, m : m + 1], b2.ap()[m * 128:(m + 1) * 128])
            b3sb = wpool.tile([E, 1], f32, tag="b3sb")
            nc.sync.dma_start(b3sb[:], b3.ap())
            id8 = wpool.tile([E, E], f32, tag="id8")
            nc.sync.dma_start(id8[:], ident.ap())

            xpf = wpool.tile([E, BC], f32, tag="xpf")      # probs, fp32
            nc.sync.dma_start(xpf[:], xp.ap())
            xpb = wpool.tile([E, BC], bf16, tag="xpb")     # probs, bf16
            nc.gpsimd.dma_start(xpb[:], xp.ap())

            # ---- main loop over batch tiles ----
            for t in range(nt):
                r0 = t * NB
                rows = slice(r0, r0 + NB)

                # cast this tile's reprs to bf16 (DRAM -> DRAM)
                nc.gpsimd.dma_start(xbf.ap()[:, rows, :], xr.ap()[:, rows, :])

                # transpose to feature-major [128 feat, NB batch]
                xt = xtp.tile([128, KC, NB], bf16, tag="xt")
                for k in range(KC):
                    e, dc = divmod(k, D // 128)
                    nc.sync.dma_start(
                        xt[:, k, :],
                        xbf.ap()[e, rows, dc * 128:(dc + 1) * 128],
                        transpose=True)

                # layer 1: h1T[j] = relu(W1_chunk.T @ xt + b1)
                h1t = h1p.tile([128, HT, NB], bf16, tag="h1t")
                for j in range(HT):
                    ps = ps1p.tile([128, NB], f32, tag="ps1")
                    for k in range(KC):
                        nc.tensor.matmul(
                            ps[:], w1sb[:, k, j * 128:(j + 1) * 128],
                            xt[:, k, :], start=(k == 0), stop=False)
                    nc.tensor.matmul(
                        ps[:], w1pr[:, j * 128:(j + 1) * 128],
                        xpb[:, rows], start=False, stop=True)
                    nc.scalar.activation(
                        h1t[:, j, :], ps[:],
                        mybir.ActivationFunctionType.Relu,
                        bias=b1sb[:, j : j + 1], scale=1.0)

                # layer 2: h2T[m] = relu(W2_chunk.T @ h1T + b2)
                h2t = h2p.tile([128, H2T, NB], bf16, tag="h2t")
                for m in range(H2T):
                    ps = ps2p.tile([128, NB], f32, tag="ps2")
                    for j in range(HT):
                        nc.tensor.matmul(
                            ps[:], w2sb[:, j, m * 128:(m + 1) * 128],
                            h1t[:, j, :], start=(j == 0), stop=(j == HT - 1))
                    nc.scalar.activation(
                        h2t[:, m, :], ps[:],
                        mybir.ActivationFunctionType.Relu,
                        bias=b2sb[:, m : m + 1], scale=1.0)

                # layer 3: zT = W3.T @ h2T + b3   [E, NB]
                ps3 = ps3p.tile([E, NB], f32, tag="ps3")
                for m in range(H2T):
                    nc.tensor.matmul(
                        ps3[:], w3sb[:, m, :], h2t[:, m, :],
                        start=(m == 0), stop=(m == H2T - 1))
                zsb = zp.tile([E, NB], f32, tag="zsb")
                nc.scalar.activation(
                    zsb[:], ps3[:], mybir.ActivationFunctionType.Identity,
                    bias=b3sb[:, 0:1], scale=1.0)

                # epilogue per 128-row sub-tile
                gw_acc = op.tile([128, NS, E], f32, tag="gw_acc")
                wp_acc = op.tile([128, NS], f32, tag="wp_acc")
                for s in range(NS):
                    c0 = s * 128
                    psz = pszp.tile([128, E], f32, tag="psz")
                    nc.tensor.transpose(psz[:], zsb[:, c0:c0 + 128], id8[:])
                    psp = pspp.tile([128, E], f32, tag="psp")
                    nc.tensor.transpose(
                        psp[:], xpf[:, r0 + c0 : r0 + c0 + 128], id8[:])

                    ex = ep.tile([128, E], f32, tag="ex")
                    nc.scalar.activation(
                        ex[:], psz[:], mybir.ActivationFunctionType.Exp)
                    amt = ep.tile([128, E], f32, tag="amt")
                    nc.sync.dma_start(amt[:], am.ap()[r0 + c0 : r0 + c0 + 128, :])

                    gm = ep.tile([128, E], f32, tag="gm")
                    ssum = ep.tile([128, 1], f32, tag="ssum")
                    nc.vector.tensor_mul(gm[:], ex[:], amt[:])
                    nc.vector.reduce_sum(ssum[:], gm[:],
                                         axis=mybir.AxisListType.X)
                    nc.vector.tensor_scalar_add(ssum[:], ssum[:], 1e-8)
                    rcp = ep.tile([128, 1], f32, tag="rcp")
                    nc.vector.reciprocal(rcp[:], ssum[:])
                    nc.vector.tensor_scalar_mul(gw_acc[:, s, :], gm[:], rcp[:])
                    junk = ep.tile([128, E], f32, tag="junk")
                    nc.vector.tensor_mul(junk[:], gw_acc[:, s, :], psp[:])
                    nc.vector.reduce_sum(wp_acc[:, s : s + 1], junk[:],
                                         axis=mybir.AxisListType.X)
                    nc.sync.dma_start(
                        wp_o.ap()[r0 + c0 : r0 + c0 + 128, 0], wp_acc[:, s : s + 1])

                nc.sync.dma_start(
                    gw_o.ap()[rows, :].rearrange("(s p) e -> p s e", p=128),
                    gw_acc[:])

    nc.finalize()
    return nc


def kernel(expert_reprs, expert_probs, active_mask, W1, b1, W2, b2, W3, b3):
    if "nc" not in _CACHE:
        _CACHE["nc"] = _build_program()
    nc = _CACHE["nc"]

    ident = np.eye(E, dtype=np.float32)
    in_maps = []
    for c in range(NCORES):
        sl = slice(c * BC, (c + 1) * BC)
        in_maps.append({
            "xr": np.ascontiguousarray(expert_reprs[:, sl, :], dtype=np.float32),
            "xp": np.ascontiguousarray(expert_probs[:, sl, 0], dtype=np.float32),
            "am": np.ascontiguousarray(active_mask[sl], dtype=np.float32),
            "w1": np.asarray(W1, dtype=np.float32),
            "b1": np.asarray(b1, dtype=np.float32),
            "w2": np.asarray(W2, dtype=np.float32),
            "b2": np.asarray(b2, dtype=np.float32),
            "w3": np.asarray(W3, dtype=np.float32),
            "b3": np.asarray(b3, dtype=np.float32),
            "ident": ident,
        })

    res = run_bass_kernel_spmd(nc, in_maps, core_ids=list(range(NCORES)))
    wp = np.concatenate([res.results[c]["wp"] for c in range(NCORES)], axis=0)
    gw = np.concatenate([res.results[c]["gw"] for c in range(NCORES)], axis=0)
    return wp.astype(np.float32), gw.astype(np.float32)
